# revision 1
# baseline (speedup 1.0000x reference)
"""Trainium2 Bass kernel for nn_BlockLinear forward.

Computes y[b, o] = sum_k exp(log_weight[o, k]) * x[b, o*K + k]
for x [16384, 8192] fp32, log_weight [1024, 8] fp32.

Strategy: data-parallel over batch across 8 NeuronCores (2048 rows each).
Per core, 16 tiles of [128, 8192] stream through SBUF.  The fused
multiply + grouped-reduce runs as ONE custom DVE op per tile:

    S[p, t] = cumsum_t(x[p, t] * w[t])        (scan(ADD, Src0*Src1), II=1)

The scan is SEGMENTED in hardware: a hand-grafted SUB_DIM_DONE step
state in the uop FSM drops the CURR feedback for exactly one element at
every page boundary of in0's [P, G, K] access pattern, resetting the
running sum per group of K (verified on HW: zero per-page overhead,
8690ns for 8192 elems, rel err 1.1e-7).  The OUTPUT access pattern has
innermost stride 0 over each group: all K writes land on one address
and the last (the completed group sum) survives — so one instruction
per tile produces the finished y tile, contiguous and compact.

Why custom: the native tensor_tensor_scan is II=2 (its recurrence
chains two ALU stages); a single-stage ADD recurrence over the stage-0
product runs at 1 element/cycle.  Loads ride the Sync HWDGE queue and
stores the ScalarE HWDGE queue so store sem-waits never block load
issues (HWDGE is FIFO per issuing engine).

Per tile: 8.7us DVE vs 10-14.9us DMA (4.5 MiB; rate depends on
neighbor-core HBM phase) -> memory-bound.  Buffering (4 x-tile bufs +
a dedicated tail-quarter pool), a quarter-split w broadcast gating
quarter-scans of the first tile (Tile deps are AP-range-based), and
the w load riding first on the Sync HWDGE FIFO keep the DMA stream
continuous end to end; first scan starts at ~25us, steady cadence
tracks the DMA at ~10.9us/tile, tail quarters at 2.2us.  Measured on
the 8 axon trn2 cores: 201.5-237us across runs depending on HBM
contention phase (final config validated at 212.5us), scale-relative
error 1.1e-7.
"""

import numpy as np

B = 16384
IN_F = 8192
OUT_F = 1024
K = 8
N_CORES = 8
P = 128

_CACHE = {}

_OP_NAME = "SEGSUM_MUL_SCAN_ANT"
_OP2_NAME = "SEGSUM8_RESET_ANT"


def _build_seg_uops(spec, ver):
    """Lower scan(ADD, Src0*Src1) then graft a SUB_DIM_DONE step state that
    drops the CURR feedback for one element — an exact segmented scan that
    resets at every page boundary of in0's [P, S, N] access pattern."""
    import dataclasses

    from concourse import dve_spec as ds
    from concourse.dve_uop import Trigger

    spec_h = ds._hoist_stream_invariant_ops(spec)
    scans = ds._collect(spec_h.body, ds.Scan)
    latches = ds._collect(spec_h.body, ds.Latch)
    placement = ds._build_placement(
        spec_h, scans, ds.N_STAGES[ver], ds.N_LANES[ver]
    )
    states = ds._build_state_machine(spec_h, scans, latches, placement)
    d = placement.node_stage[scans[0]]
    steady_idx = len(states) - 1
    step_idx = steady_idx + 1
    steady = states[steady_idx]
    states[steady_idx] = dataclasses.replace(
        steady,
        trigger=(Trigger.SRC_TENSOR_DONE, Trigger.SUB_DIM_DONE, Trigger.NONE),
        next=(0, step_idx, 0),
    )
    states.append(
        dataclasses.replace(
            steady,
            overrides={
                **steady.overrides,
                d: ds._Stage(ds.AluOp.BYPASS, scans[0].expr),
            },
            trigger=(Trigger.SRC_TENSOR_DONE, Trigger.SUB_DIM_DONE, Trigger.COUNT),
            next=(0, step_idx, steady_idx),
            repeat=1,
        )
    )
    uops = [ds._assemble(st) for st in states]
    for u in uops:
        u.validate(ver)
    return uops


def _register_seg_op():
    """Register the segmented multiply-scan (page-reset) custom DVE op."""
    import dataclasses

    from concourse import dve_ops
    from concourse.dve_spec import AluOp, Spec, Src0, Src1, scan
    from concourse.dve_uop import DveOpSpec

    for op in dve_ops.OPS:
        if op.name == _OP2_NAME:
            return op

    def _ref(in0, in1, s0, s1, imm2):
        p = (
            np.asarray(in0, np.float32)
            * np.asarray(in1, np.float32).reshape(np.asarray(in0).shape)
        ).astype(np.float32)
        return np.cumsum(p, axis=-1, dtype=np.float32)

    spec = Spec(body=scan(AluOp.ADD, Src0 * Src1), reference=_ref)

    @dataclasses.dataclass(frozen=True)
    class _SegDveOp(dve_ops.DveOp):
        def compile(self, ver):
            key = (self.name, ver)
            cached = dve_ops._COMPILE_CACHE.get(key)
            if cached is not None:
                return cached
            result = DveOpSpec(
                name=self.name,
                opcode=dve_ops.get_dve_sub_opcode(self.name),
                uops=_build_seg_uops(self.spec, ver),
                rd1_en=True,
            )
            got = result.sha(ver)
            if self.uops_sha.get(ver) != got:
                raise ValueError(f"{self.name}: uop drift {got}")
            dve_ops._COMPILE_CACHE[key] = result
            return result

    row = dve_ops._CUSTOM_DVE_ROW_BASE + len(dve_ops.OPS)
    shas = {}
    for ver in ("v3", "v4"):
        s = DveOpSpec(
            name=_OP2_NAME, opcode=row, uops=_build_seg_uops(spec, ver), rd1_en=True
        )
        shas[ver] = s.sha(ver)
    op = _SegDveOp(_OP2_NAME, spec, subdim=True, uops_sha=shas)
    dve_ops.OPS.append(op)
    dve_ops.CUSTOM_DVE_SPECS[_OP2_NAME] = spec
    dve_ops._SUB_OPCODE_FOR_NAME[_OP2_NAME] = row
    return op


def _register_custom_op():
    """Register scan(ADD, Src0*Src1) as a custom DVE op (runtime-local)."""
    from concourse import dve_ops
    from concourse.dve_spec import AluOp, Spec, Src0, Src1, _has_src1, lower, scan
    from concourse.dve_uop import DveOpSpec

    for op in dve_ops.OPS:
        if op.name == _OP_NAME:
            return op

    def _ref(in0, in1, s0, s1, imm2):
        p = (np.asarray(in0, np.float32) * np.asarray(in1, np.float32)).astype(
            np.float32
        )
        shp = p.shape
        return (
            np.cumsum(p.reshape(shp[0], -1), axis=1, dtype=np.float32).reshape(shp)
        )

    spec = Spec(body=scan(AluOp.ADD, Src0 * Src1), reference=_ref)
    row = dve_ops._CUSTOM_DVE_ROW_BASE + len(dve_ops.OPS)
    shas = {}
    for ver in ("v3", "v4"):
        s = DveOpSpec(
            name=_OP_NAME, opcode=row, uops=lower(spec, ver=ver), rd1_en=_has_src1(spec)
        )
        shas[ver] = s.sha(ver)
    op = dve_ops.DveOp(_OP_NAME, spec, subdim=False, uops_sha=shas)
    dve_ops.OPS.append(op)
    dve_ops.CUSTOM_DVE_SPECS[_OP_NAME] = spec
    dve_ops._SUB_OPCODE_FOR_NAME[_OP_NAME] = row
    return op


def _build(b_shard, in_f, out_f, n_cores, x_bufs=4, halves=4, n_prologue=0, tail_quarters=4):
    """Build + compile the per-core Bass module (SPMD across n_cores)."""
    from concourse import bacc, tile, mybir

    op = _register_custom_op()
    op2 = _register_seg_op()

    k = K
    n_tiles = b_shard // P
    hw = in_f // halves  # half-tile width (multiple of K)
    hy = hw // k
    f32 = mybir.dt.float32

    nc = bacc.Bacc(
        "TRN2",
        target_bir_lowering=False,
        debug=False,
        enable_asserts=True,
        num_devices=n_cores,
    )
    x_d = nc.dram_tensor("x", [b_shard, in_f], f32, kind="ExternalInput")
    w_d = nc.dram_tensor("w", [1, in_f], f32, kind="ExternalInput")
    y_d = nc.dram_tensor("y", [b_shard, out_f], f32, kind="ExternalOutput")

    with tile.TileContext(nc) as tc:
        with (
            tc.tile_pool(name="consts", bufs=1) as cpool,
            tc.tile_pool(name="work", bufs=x_bufs) as pool,
            tc.tile_pool(name="outs", bufs=3) as ypool,
            tc.tile_pool(name="tailq", bufs=4) as qpool,
        ):
            wb = cpool.tile([P, in_f], f32, tag="w")
            # w first in the Sync HWDGE FIFO: its 32KB completes ~5us
            # earlier than via SWDGE (GpSimd's preamble delays emission),
            # and it only displaces x0's issue by ~0.7us.
            nc.sync.dma_start(out=wb[0:1, :], in_=w_d[:])
            for h in range(halves):
                nc.gpsimd.partition_broadcast(
                    wb[:, h * hw : (h + 1) * hw], wb[0:1, h * hw : (h + 1) * hw]
                )
            def chunk(i, xap, c0, cw):
                """Process columns [c0, c0+cw) of row-block i from AP xap."""
                rows = slice(i * P, (i + 1) * P)
                cg = cw // k  # groups in this chunk
                # One instruction per chunk: segmented multiply-scan with a
                # hardware page reset (SUB_DIM_DONE step state) over in0's
                # [P, cg, K] access pattern.  The out AP has innermost
                # stride 0 over each group's K elements, so the last write
                # (the completed group sum) survives, laid out contiguously.
                yt = ypool.tile([P, cg], f32, tag="s")
                y_view = yt[:].rearrange("p (g o) -> p g o", o=1).broadcast_to(
                    [P, cg, k]
                )
                nc.vector._custom_dve(
                    op2,
                    out=y_view,
                    in0=xap.rearrange("p (g kk) -> p g kk", kk=k),
                    in1=wb[:, c0 : c0 + cw],
                )
                # y stores ride the ScalarE HWDGE queue so their semaphore
                # waits never block the x-load issue stream (HWDGE is FIFO
                # per issuing engine).
                nc.scalar.dma_start(
                    out=y_d[rows, c0 // k : (c0 + cw) // k], in_=yt[:]
                )

            for i in range(n_tiles):
                rows = slice(i * P, (i + 1) * P)
                if i == n_tiles - 1 and tail_quarters > 1:
                    # split the final tile so the post-stream tail is short
                    qw = in_f // tail_quarters
                    for q in range(tail_quarters):
                        xt = qpool.tile([P, qw], f32, tag="xq")
                        nc.sync.dma_start(
                            out=xt[:], in_=x_d[rows, q * qw : (q + 1) * qw]
                        )
                        chunk(i, xt[:], q * qw, qw)
                else:
                    if i < n_prologue:
                        # dedicated startup buffers: extra DMA runway at start
                        xt = cpool.tile([P, in_f], f32, tag=f"xpro{i}")
                    else:
                        xt = pool.tile([P, in_f], f32, tag="x")
                    nc.sync.dma_start(out=xt[:], in_=x_d[rows, :])
                    if i == 0 and halves > 1:
                        # quarter-scans against matching wb ranges: each
                        # gates on its own partial broadcast, starting
                        # compute ~9us earlier (no extra bytes moved)
                        for q in range(halves):
                            chunk(i, xt[:, q * hw : (q + 1) * hw], q * hw, hw)
                    else:
                        chunk(i, xt[:], 0, in_f)
    nc.compile()
    return nc


def _prep_weights(log_weight, out_f, k):
    w = np.exp(np.asarray(log_weight, np.float64)).reshape(1, -1)  # [1, out_f*k]
    return np.ascontiguousarray(w, dtype=np.float32)


def kernel(x, log_weight):
    from concourse import bass_utils

    x = np.ascontiguousarray(np.asarray(x, dtype=np.float32))
    assert x.shape == (B, IN_F), x.shape
    b_shard = B // N_CORES

    if "nc" not in _CACHE:
        _CACHE["nc"] = _build(b_shard, IN_F, OUT_F, N_CORES)
    nc = _CACHE["nc"]

    wb = _prep_weights(log_weight, OUT_F, K)
    in_maps = [
        {"x": x[i * b_shard : (i + 1) * b_shard], "w": wb}
        for i in range(N_CORES)
    ]
    res = bass_utils.run_bass_kernel_spmd(nc, in_maps, core_ids=list(range(N_CORES)))
    y = np.concatenate([res.results[i]["y"] for i in range(N_CORES)], axis=0)
    return y



# revision 3
# speedup vs baseline: 1.3172x; 1.3172x over previous
"""Trainium2 Bass kernel for nn_BlockLinear forward — PE/matmul version.

Computes y[b, o] = sum_k exp(log_weight[o, k]) * x[b, o*K + k]
for x [16384, 8192] fp32, log_weight [1024, 8] fp32.

Strategy: data-parallel over batch across 8 NeuronCores (2048 rows each).
The host pre-transposes each core's x shard to x^T [8192, 2048] fp16 and
folds exp(log_weight) into 64 sparse stationary blocks S_c [128, 128]
(S_c[p, m] = w[o, p%8] iff m == 16*(c%8) + p//8, the block-diagonal
structure of BlockLinear restricted to in-features [128c, 128c+128)).

Per core the kernel is then a pure matmul stream on the PE:

    y^T[128t:128t+128, bw] = sum_{j=0..7} S_{8t+j}.T @ x^T[chunk 8t+j, bw]

accumulated in PSUM over the 8 chunk-matmuls (start=j==0, stop=j==7),
one [128, 512] PSUM bank per (t, b) — outputs land 128-partition dense,
so ACT evacuates fp32->fp16 in one 512-cycle copy per bank.

Why this beats the DVE segmented-scan baseline (212.5us): the scan kept
x in fp32 (custom DVE ops have no 2x 16-bit mode, so fp16 could not pay
for itself), pinning DMA at 72 MiB/core.  The PE contracts along the
partition dim at 1 column/cycle regardless of stationary sparsity, so
fp16 x^T halves the dominant read traffic: 32 MiB x + 4 MiB y = 36 MiB
per core.  The 16 shared DMA engines move 4KB-descriptor streams at
~20.7-24.4 B/ns each depending on HBM contention phase, so the stream
is ~97-120us; PE busy is ~70us, ACT ~14us, DVE ~2us — all hidden under
DMA.  fp16 quantization of x, w, and y gives rel err 5.2e-4 against the
fp32 reference (tolerance 2e-2).

Trace-driven details (all verified on the axon trn2 cores):
- Stores are batched per output group into one [128, 2048] DMA with 4KB
  descriptors.  512-column stores (1KB descriptors) are desc-gen-bound
  at ~6ns/descriptor -> ~170 B/ns per queue and dribble at the tail.
- The stationary uploads as a 256KB dense-window table and expands
  on the idle DVE (memset + 8 strided scatters); uploading the 2MiB
  sparse form would cost ~5us of DMA-engine time.
- The last group's PSUM evacuation splits across DVE+ACT and its store
  splits in two so the post-stream tail is ~MM + copy + half-store
  (measured -2.7us).
- HWDGE queues cannot start before ~8.7us (NEFF preamble); instruction
  -fetch DMA runs [2.5, 5.6]us.  Mixed-K accumulation groups (K=128
  then K=64 partials into one PSUM bank) wedge the device — avoided.
- x loads ride Sync HWDGE; dw + y stores ride ScalarE HWDGE (only SP
  and Activation have HWDGE queues on this target).

Measured 112.1-130.2us across HBM phases (baseline same-day: 251us).
"""

import numpy as np

B = 16384
IN_F = 8192
OUT_F = 1024
K = 8
N_CORES = 8
P = 128

_CACHE = {}


def _build(b_shard, in_f, out_f, n_cores, x_bufs=16, psum_bufs=8, y_bufs=4, nb=4):
    """Build + compile the per-core Bass module (SPMD across n_cores)."""
    from concourse import bacc, tile, mybir

    f16 = mybir.dt.float16
    f32 = mybir.dt.float32
    n_chunks = in_f // P  # 64 in-feature chunks of 128
    n_groups = n_chunks // 8  # 8 output groups of 128
    bw = b_shard // nb  # batch columns per PSUM bank (512 fp32 = 1 bank)

    nc = bacc.Bacc(
        "TRN2",
        target_bir_lowering=False,
        debug=False,
        enable_asserts=True,
        num_devices=n_cores,
    )
    xt_d = nc.dram_tensor("xt", [in_f, b_shard], f16, kind="ExternalInput")
    d_d = nc.dram_tensor("d", [P, n_chunks * 16], f16, kind="ExternalInput")
    y_d = nc.dram_tensor("y", [out_f, b_shard], f16, kind="ExternalOutput")

    with tile.TileContext(nc) as tc:
        with (
            tc.tile_pool(name="consts", bufs=1) as cpool,
            tc.tile_pool(name="xs", bufs=x_bufs) as xpool,
            tc.tile_pool(name="ys", bufs=y_bufs) as ypool,
            tc.tile_pool(name="ps", bufs=psum_bufs, space="PSUM") as ppool,
        ):
            # Stationary: upload only the 256KB dense-window form
            # D[8g+k, 16c+m16] = S_c[8g+k, 16*(c%8)+m16] (the 2MiB sparse
            # form is 7/8 zeros — uploading it costs ~5us of DMA-engine
            # time).  Expand on the idle DVE: memset + 8 strided scatters
            # st[:, 1024t+144j+m16] = D[:, (8t+j)*16+m16], all partition-
            # aligned (the BIR verifier rejects partition bases not in
            # {0,32,64,96}).
            st = cpool.tile([P, n_chunks * P], f16, tag="s")
            dw = cpool.tile([P, n_chunks * 16], f16, tag="dw")
            nc.scalar.dma_start(out=dw[:], in_=d_d[:])
            nc.vector.memset(st[:].bitcast(mybir.dt.uint32), 0)
            st_v = st[:].rearrange("p (t r) -> p t r", t=8)
            dw_v = dw[:].rearrange("p (t j z) -> p t j z", t=8, j=8)
            for j in range(8):
                nc.vector.tensor_scalar_mul(
                    st_v[:, :, 144 * j : 144 * j + 16], dw_v[:, :, j, :], 1.0
                )

            for t in range(n_groups):
                pss = [
                    ppool.tile([P, bw], f32, tag="ps", name=f"ps_{t}_{b}")
                    for b in range(nb)
                ]
                last = t == n_groups - 1
                xts = []
                for j in range(8):
                    c = t * 8 + j
                    xt = xpool.tile([P, b_shard], f16, tag="x", name=f"x_{c}")
                    nc.sync.dma_start(out=xt[:], in_=xt_d[c * P : (c + 1) * P, :])
                    xts.append(xt)
                # j outer / b inner: each x tile is fully consumed by its
                # 4 matmuls on arrival and recycles promptly.
                for j in range(8):
                    c = t * 8 + j
                    for b in range(nb):
                        nc.tensor.matmul(
                            pss[b][:],
                            lhsT=st[:, c * P : (c + 1) * P],
                            rhs=xts[j][:, b * bw : (b + 1) * bw],
                            start=(j == 0),
                            stop=(j == 7),
                        )
                # Stage all 4 banks into one [128, 2048] tile and store the
                # group's full y^T rows in ONE DMA: 4KB descriptors, same
                # efficiency as the loads.  Per-bank 512-col stores would be
                # 1KB descriptors, which are desc-gen-limited (~170 B/ns per
                # queue) and dribble at the tail.
                yt = ypool.tile([P, b_shard], f16, tag="y", name=f"y_{t}")
                if last:
                    # Copies split across DVE + ACT, store split in two, so
                    # the post-stream tail is a single copy + half-store.
                    nc.vector.tensor_scalar_mul(yt[:, 0 * bw : 1 * bw], pss[0][:], 1.0)
                    nc.scalar.copy(out=yt[:, 1 * bw : 2 * bw], in_=pss[1][:])
                    nc.scalar.dma_start(
                        out=y_d[t * P : (t + 1) * P, 0 : 2 * bw], in_=yt[:, 0 : 2 * bw]
                    )
                    nc.vector.tensor_scalar_mul(yt[:, 2 * bw : 3 * bw], pss[2][:], 1.0)
                    nc.scalar.copy(out=yt[:, 3 * bw : 4 * bw], in_=pss[3][:])
                    nc.scalar.dma_start(
                        out=y_d[t * P : (t + 1) * P, 2 * bw :], in_=yt[:, 2 * bw :]
                    )
                else:
                    for b in range(nb):
                        nc.scalar.copy(out=yt[:, b * bw : (b + 1) * bw], in_=pss[b][:])
                    nc.scalar.dma_start(out=y_d[t * P : (t + 1) * P, :], in_=yt[:])
    nc.compile()
    return nc


def _prep_inputs(x, log_weight, b_shard):
    """Host-side layout prep: dense weight table + per-core x^T fp16 shards."""
    w = np.exp(np.asarray(log_weight, np.float64)).astype(np.float32)  # [1024, 8]
    # D[8g+k, 16c+g] = w[128*(c//8) + 16*(c%8) + g, k], zero elsewhere
    g_ix = np.arange(16)
    c_ix = np.arange(64)
    o_idx = 128 * (c_ix[None, :] // 8) + 16 * (c_ix[None, :] % 8) + g_ix[:, None]
    d4 = np.zeros((16, 8, 64, 16), np.float32)
    d4[g_ix[:, None], :, c_ix[None, :], g_ix[:, None]] = w[o_idx]
    dmat = np.ascontiguousarray(d4.reshape(P, 1024), dtype=np.float16)

    x16 = np.asarray(x, np.float32).astype(np.float16)
    in_maps = []
    for i in range(N_CORES):
        xt = np.ascontiguousarray(x16[i * b_shard : (i + 1) * b_shard].T)
        in_maps.append({"xt": xt, "d": dmat})
    return in_maps


def kernel(x, log_weight, _trace_dir=None):
    from concourse import bass_utils

    b_shard = B // N_CORES
    if "nc" not in _CACHE:
        _CACHE["nc"] = _build(b_shard, IN_F, OUT_F, N_CORES)
    nc = _CACHE["nc"]

    in_maps = _prep_inputs(x, log_weight, b_shard)
    kwargs = {}
    if _trace_dir is not None:
        kwargs = {"trace": True, "tmpdir": _trace_dir}
    res = bass_utils.run_bass_kernel_spmd(
        nc, in_maps, core_ids=list(range(N_CORES)), **kwargs
    )
    _CACHE["last_res"] = res
    y = np.empty((B, OUT_F), np.float32)
    for i in range(N_CORES):
        y[i * b_shard : (i + 1) * b_shard] = res.results[i]["y"].T.astype(np.float32)
    return y


# revision 4
# speedup vs baseline: 1.4285x; 1.0845x over previous
"""Trainium2 Bass kernel for nn_BlockLinear forward — hybrid fp16/fp8 PE version.

Computes y[b, o] = sum_k exp(log_weight[o, k]) * x[b, o*K + k]
for x [16384, 8192] fp32, log_weight [1024, 8] fp32.

Data-parallel over batch across 8 NeuronCores (2048 rows each).  Builds
on the fp16 PE kernel (112us), cutting DMA bytes further with weight-
aware mixed precision: for each output o, the 2 features with the
largest w = exp(log_weight) stay fp16 ("hot"), the remaining 6 go fp8
e4m3 ("cold").  Quantization error is dominated by w-amplified terms,
so routing only the top-2 weights per output through fp16 gives rel err
6.7e-3 (measured on the fixed-seed inputs; tolerance 2e-2) while the
x stream shrinks 33.5 MB -> 21 MB/core (+4.2 MB y out).

The host permutes features into per-output-sorted order, which makes
EVERY stationary diagonal: chunk c = 8t + j of group t holds feature
rank j of output o = 128t + p at row p, so S_c = diag(w-rank-j).  The
device builds all 64 [128,128] stationary blocks from a 16KB weight
table with one tensor_scalar_mul each against an uploaded identity
(per-partition scalar broadcast; fp16 out for hot chunks, fp8 for cold).

Per group and batch-quarter b: 2 hot fp16 matmuls accumulate in a hot
PSUM bank, 6 cold fp8 matmuls in a cold bank (groups never mix dtypes
in one accumulation bank — mixed groups wedge the device, as do
mixed-K partial accumulations).  The hot bank evacuates early via ACT
copy to an fp32 staging tile (freeing hot banks mid-group), then DVE
fuses (cold + staged hot) -> fp16 with one scalar_tensor_tensor per
bank.  Stores batch per group into one [128, 2048] DMA (4KB
descriptors; 1KB-descriptor stores are desc-gen-bound at ~6ns/desc).

Streams: hot x [2048, 2048] f16 + cold x [6144, 2048] f8 + y out
[1024, 2048] f16 = 25.2 MB/core vs 37.8 before.  x loads ride Sync
HWDGE; tables + y stores ride ScalarE HWDGE.  HWDGE queues cannot
start before ~8.7us (NEFF preamble).  PE: 256 matmuls x 512 cols +
self-loaded stationaries ~= 68us, roughly matching the stream.
"""

import numpy as np

B = 16384
IN_F = 8192
OUT_F = 1024
K = 8
N_CORES = 8
P = 128
N_HOT = 2  # hot (fp16) feature ranks per output; rest are fp8

_CACHE = {}


def _build(b_shard, in_f, out_f, n_cores, x_bufs=10, c_bufs=14, y_bufs=3, nb=4):
    """Build + compile the per-core Bass module (SPMD across n_cores)."""
    from concourse import bacc, tile, mybir

    f16 = mybir.dt.float16
    f8 = mybir.dt.float8e4
    f32 = mybir.dt.float32
    n_groups = out_f // P  # 8 output groups of 128
    n_cold = 8 - N_HOT
    bw = b_shard // nb  # batch columns per PSUM bank (512 fp32 = 1 bank)

    nc = bacc.Bacc(
        "TRN2",
        target_bir_lowering=False,
        debug=False,
        enable_asserts=True,
        num_devices=n_cores,
    )
    xh_d = nc.dram_tensor("xh", [n_groups * N_HOT * P, b_shard], f16, kind="ExternalInput")
    xc_d = nc.dram_tensor("xc", [n_groups * n_cold * P, b_shard], f8, kind="ExternalInput")
    v_d = nc.dram_tensor("v", [P, 8 * n_groups], f32, kind="ExternalInput")
    id_d = nc.dram_tensor("idm", [P, P], f16, kind="ExternalInput")
    y_d = nc.dram_tensor("y", [out_f, b_shard], f16, kind="ExternalOutput")

    with tile.TileContext(nc) as tc:
        with (
            tc.tile_pool(name="consts", bufs=1) as cpool,
            tc.tile_pool(name="xh", bufs=x_bufs) as hpool,
            tc.tile_pool(name="xc", bufs=c_bufs) as cxpool,
            tc.tile_pool(name="ys", bufs=y_bufs) as ypool,
            tc.tile_pool(name="yh", bufs=2) as spool,
            tc.tile_pool(name="ps", bufs=4, space="PSUM") as ppool,
        ):
            ident = cpool.tile([P, P], f16, tag="id")
            vt = cpool.tile([P, 8 * n_groups], f32, tag="v")
            st16 = cpool.tile([P, n_groups * N_HOT * P], f16, tag="s16")
            st8 = cpool.tile([P, n_groups * n_cold * P], f8, tag="s8")
            nc.scalar.dma_start(out=ident[:], in_=id_d[:])
            nc.scalar.dma_start(out=vt[:], in_=v_d[:])
            for t in range(n_groups):
                last = t == n_groups - 1
                # Group t's diagonal stationaries: S = ident * v[:, c]
                # broadcast per partition (v[p, 8t+j] = rank-j weight of
                # output 128t+p).  Emitted INSIDE the loop so the DVE FIFO
                # interleaves builds with the PSUM-freeing fuse ops — built
                # up front, every fuse queues behind ~15us of builds and the
                # PE stalls on PSUM recycling.
                from concourse import mybir as mb

                id_h = ident[:].rearrange("p (x m) -> p x m", x=1).broadcast_to([P, N_HOT, P])
                v_h = (
                    vt[:, 8 * t : 8 * t + N_HOT]
                    .rearrange("p (j x) -> p j x", x=1)
                    .broadcast_to([P, N_HOT, P])
                )
                nc.vector.scalar_tensor_tensor(
                    st16[:, t * N_HOT * P : (t + 1) * N_HOT * P].rearrange(
                        "p (j m) -> p j m", j=N_HOT
                    ),
                    id_h, 1.0, v_h, mb.AluOpType.mult, mb.AluOpType.mult,
                )
                id_c = ident[:].rearrange("p (x m) -> p x m", x=1).broadcast_to([P, n_cold, P])
                v_c = (
                    vt[:, 8 * t + N_HOT : 8 * t + 8]
                    .rearrange("p (j x) -> p j x", x=1)
                    .broadcast_to([P, n_cold, P])
                )
                nc.vector.scalar_tensor_tensor(
                    st8[:, t * n_cold * P : (t + 1) * n_cold * P].rearrange(
                        "p (j m) -> p j m", j=n_cold
                    ),
                    id_c, 1.0, v_c, mb.AluOpType.mult, mb.AluOpType.mult,
                )
                hbs = [
                    ppool.tile([P, bw], f32, tag="ph", name=f"ph_{t}_{b}")
                    for b in range(nb)
                ]
                cbs = [
                    ppool.tile([P, bw], f32, tag="pc", name=f"pc_{t}_{b}")
                    for b in range(nb)
                ]
                # loads in consumption order: hot ranks 0..1, then cold 0..5
                hts, cts = [], []
                for j in range(N_HOT):
                    r = (t * N_HOT + j) * P
                    xt = hpool.tile([P, b_shard], f16, tag="xh", name=f"xh_{t}_{j}")
                    nc.sync.dma_start(out=xt[:], in_=xh_d[r : r + P, :])
                    hts.append(xt)
                for j in range(n_cold):
                    r = (t * n_cold + j) * P
                    xt = cxpool.tile([P, b_shard], f8, tag="xc", name=f"xc_{t}_{j}")
                    nc.sync.dma_start(out=xt[:], in_=xc_d[r : r + P, :])
                    cts.append(xt)
                # hot accumulation (fp16 x fp16), 2 matmuls per bank
                for j in range(N_HOT):
                    hc = t * N_HOT + j
                    for b in range(nb):
                        nc.tensor.matmul(
                            hbs[b][:],
                            lhsT=st16[:, hc * P : (hc + 1) * P],
                            rhs=hts[j][:, b * bw : (b + 1) * bw],
                            start=(j == 0),
                            stop=(j == N_HOT - 1),
                        )
                # evacuate hot banks early to fp32 staging (frees them for
                # the next group while cold matmuls still run)
                ysb = spool.tile([P, b_shard], f32, tag="yh", name=f"yh_{t}")
                for b in range(nb):
                    nc.scalar.copy(out=ysb[:, b * bw : (b + 1) * bw], in_=hbs[b][:])
                # cold accumulation (fp8 x fp8), 6 matmuls per bank
                for j in range(n_cold):
                    cc = t * n_cold + j
                    for b in range(nb):
                        nc.tensor.matmul(
                            cbs[b][:],
                            lhsT=st8[:, cc * P : (cc + 1) * P],
                            rhs=cts[j][:, b * bw : (b + 1) * bw],
                            start=(j == 0),
                            stop=(j == n_cold - 1),
                        )
                # free cold banks fast via ACT copies to fp32 staging, then
                # fuse cold + hot -> fp16 on DVE off the PSUM critical path
                csb = spool.tile([P, b_shard], f32, tag="yc", name=f"yc_{t}")
                yt = ypool.tile([P, b_shard], f16, tag="y", name=f"y_{t}")
                for b in range(nb):
                    nc.scalar.copy(out=csb[:, b * bw : (b + 1) * bw], in_=cbs[b][:])
                for b in range(nb):
                    nc.vector.scalar_tensor_tensor(
                        yt[:, b * bw : (b + 1) * bw],
                        csb[:, b * bw : (b + 1) * bw],
                        1.0,
                        ysb[:, b * bw : (b + 1) * bw],
                        mb.AluOpType.mult,
                        mb.AluOpType.add,
                    )
                    if last and b == 1:
                        nc.scalar.dma_start(
                            out=y_d[t * P : (t + 1) * P, 0 : 2 * bw],
                            in_=yt[:, 0 : 2 * bw],
                        )
                if last:
                    nc.scalar.dma_start(
                        out=y_d[t * P : (t + 1) * P, 2 * bw :], in_=yt[:, 2 * bw :]
                    )
                else:
                    nc.scalar.dma_start(out=y_d[t * P : (t + 1) * P, :], in_=yt[:])
    nc.compile()
    return nc


def _prep_inputs(x, log_weight, b_shard):
    """Host-side: per-output weight sort, permutation gather, mixed casts."""
    from concourse import mybir

    f8np = mybir.dt.np(mybir.dt.float8e4)
    w = np.exp(np.asarray(log_weight, np.float64)).astype(np.float32)  # [1024, 8]
    ordk = np.argsort(-w, axis=1)  # [1024, 8] feature ranks per output
    o_all = np.arange(OUT_F)
    wsort = w[o_all[:, None], ordk]  # [1024, 8]
    # v[p, 8t+j] = wsort[128t+p, j]
    v = np.ascontiguousarray(
        wsort.reshape(8, P, 8).transpose(1, 0, 2).reshape(P, 64), dtype=np.float32
    )
    feat = (8 * o_all[:, None] + ordk).reshape(8, P, 8)  # [t, p, j]
    hot_idx = feat[:, :, :N_HOT].transpose(0, 2, 1).reshape(-1)  # [(t*2+j)*128+p]
    cold_idx = feat[:, :, N_HOT:].transpose(0, 2, 1).reshape(-1)
    ident = np.eye(P, dtype=np.float16)

    x16 = np.asarray(x, np.float32).astype(np.float16)
    in_maps = []
    for i in range(N_CORES):
        xt = np.ascontiguousarray(x16[i * b_shard : (i + 1) * b_shard].T)
        xh = np.ascontiguousarray(xt[hot_idx])
        xc = np.ascontiguousarray(xt[cold_idx]).astype(f8np)
        in_maps.append({"xh": xh, "xc": xc, "v": v, "idm": ident})
    return in_maps


def kernel(x, log_weight, _trace_dir=None):
    from concourse import bass_utils

    b_shard = B // N_CORES
    if "nc" not in _CACHE:
        _CACHE["nc"] = _build(b_shard, IN_F, OUT_F, N_CORES)
    nc = _CACHE["nc"]

    in_maps = _prep_inputs(x, log_weight, b_shard)
    kwargs = {}
    if _trace_dir is not None:
        kwargs = {"trace": True, "tmpdir": _trace_dir}
    res = bass_utils.run_bass_kernel_spmd(
        nc, in_maps, core_ids=list(range(N_CORES)), **kwargs
    )
    _CACHE["last_res"] = res
    y = np.empty((B, OUT_F), np.float32)
    for i in range(N_CORES):
        y[i * b_shard : (i + 1) * b_shard] = res.results[i]["y"].T.astype(np.float32)
    return y


# revision 5
# speedup vs baseline: 1.5177x; 1.0624x over previous
"""Trainium2 Bass kernel for nn_BlockLinear forward — hybrid fp16/fp8 PE version.

Computes y[b, o] = sum_k exp(log_weight[o, k]) * x[b, o*K + k]
for x [16384, 8192] fp32, log_weight [1024, 8] fp32.

Data-parallel over batch across 8 NeuronCores (2048 rows each).  Builds
on the fp16 PE kernel (112us), cutting DMA bytes further with weight-
aware mixed precision: for each output o, the 2 features with the
largest w = exp(log_weight) stay fp16 ("hot"), the remaining 6 go fp8
e4m3 ("cold").  Quantization error is dominated by w-amplified terms,
so routing only the top-2 weights per output through fp16 gives rel err
6.7e-3 (measured on the fixed-seed inputs; tolerance 2e-2) while the
x stream shrinks 33.5 MB -> 21 MB/core (+4.2 MB y out).

The host permutes features into per-output-sorted order, which makes
EVERY stationary diagonal: chunk c = 8t + j of group t holds feature
rank j of output o = 128t + p at row p, so S_c = diag(w-rank-j).  The
device builds all 64 [128,128] stationary blocks from a 16KB weight
table with one tensor_scalar_mul each against an uploaded identity
(per-partition scalar broadcast; fp16 out for hot chunks, fp8 for cold).

Per group and batch-quarter b: 2 hot fp16 matmuls accumulate in a hot
PSUM bank, 6 cold fp8 matmuls in a cold bank (groups never mix dtypes
in one accumulation bank — mixed groups wedge the device, as do
mixed-K partial accumulations).  The hot bank evacuates early via ACT
copy to an fp32 staging tile (freeing hot banks mid-group), then DVE
fuses (cold + staged hot) -> fp16 with one scalar_tensor_tensor per
bank.  Stores batch per group into one [128, 2048] DMA (4KB
descriptors; 1KB-descriptor stores are desc-gen-bound at ~6ns/desc).

Streams: hot x [2048, 2048] f16 + cold x [6144, 2048] f8 + y out
[1024, 2048] f16 = 25.2 MB/core vs 37.8 before.  x loads ride Sync
HWDGE; tables + y stores ride ScalarE HWDGE.  HWDGE queues cannot
start before ~8.7us (NEFF preamble).  PE: 256 matmuls x 512 cols +
self-loaded stationaries ~= 68us, roughly matching the stream.
"""

import numpy as np

B = 16384
IN_F = 8192
OUT_F = 1024
K = 8
N_CORES = 8
P = 128
N_HOT = 2  # hot (fp16) feature ranks per output; rest are fp8

_CACHE = {}


def _build(b_shard, in_f, out_f, n_cores, x_bufs=6, c_bufs=18, y_bufs=3, nb=4):
    """Build + compile the per-core Bass module (SPMD across n_cores)."""
    from concourse import bacc, tile, mybir

    f16 = mybir.dt.float16
    f8 = mybir.dt.float8e4
    f32 = mybir.dt.float32
    n_groups = out_f // P  # 8 output groups of 128
    n_cold = 8 - N_HOT
    bw = b_shard // nb  # batch columns per PSUM bank (512 fp32 = 1 bank)

    nc = bacc.Bacc(
        "TRN2",
        target_bir_lowering=False,
        debug=False,
        enable_asserts=True,
        num_devices=n_cores,
    )
    xh_d = nc.dram_tensor("xh", [n_groups * N_HOT * P, b_shard], f16, kind="ExternalInput")
    xc_d = nc.dram_tensor("xc", [n_groups * n_cold * P, b_shard], f8, kind="ExternalInput")
    v_d = nc.dram_tensor("v", [P, 8 * n_groups], f32, kind="ExternalInput")
    id_d = nc.dram_tensor("idm", [P, P], f16, kind="ExternalInput")
    y_d = nc.dram_tensor("y", [out_f, b_shard], f16, kind="ExternalOutput")

    with tile.TileContext(nc) as tc:
        with (
            tc.tile_pool(name="consts", bufs=1) as cpool,
            tc.tile_pool(name="xh", bufs=x_bufs) as hpool,
            tc.tile_pool(name="xc", bufs=c_bufs) as cxpool,
            tc.tile_pool(name="ys", bufs=y_bufs) as ypool,
            tc.tile_pool(name="yh", bufs=2) as spool,
            tc.tile_pool(name="ps", bufs=4, space="PSUM") as ppool,
        ):
            ident = cpool.tile([P, P], f16, tag="id")
            vt = cpool.tile([P, 8 * n_groups], f32, tag="v")
            st16 = cpool.tile([P, n_groups * N_HOT * P], f16, tag="s16")
            st8 = cpool.tile([P, n_groups * n_cold * P], f8, tag="s8")
            nc.scalar.dma_start(out=ident[:], in_=id_d[:])
            nc.scalar.dma_start(out=vt[:], in_=v_d[:])
            for t in range(n_groups):
                last = t == n_groups - 1
                # Group t's diagonal stationaries: S = ident * v[:, c]
                # broadcast per partition (v[p, 8t+j] = rank-j weight of
                # output 128t+p).  Emitted INSIDE the loop so the DVE FIFO
                # interleaves builds with the PSUM-freeing fuse ops — built
                # up front, every fuse queues behind ~15us of builds and the
                # PE stalls on PSUM recycling.
                from concourse import mybir as mb

                id_h = ident[:].rearrange("p (x m) -> p x m", x=1).broadcast_to([P, N_HOT, P])
                v_h = (
                    vt[:, 8 * t : 8 * t + N_HOT]
                    .rearrange("p (j x) -> p j x", x=1)
                    .broadcast_to([P, N_HOT, P])
                )
                nc.vector.scalar_tensor_tensor(
                    st16[:, t * N_HOT * P : (t + 1) * N_HOT * P].rearrange(
                        "p (j m) -> p j m", j=N_HOT
                    ),
                    id_h, 1.0, v_h, mb.AluOpType.mult, mb.AluOpType.mult,
                )
                id_c = ident[:].rearrange("p (x m) -> p x m", x=1).broadcast_to([P, n_cold, P])
                v_c = (
                    vt[:, 8 * t + N_HOT : 8 * t + 8]
                    .rearrange("p (j x) -> p j x", x=1)
                    .broadcast_to([P, n_cold, P])
                )
                nc.vector.scalar_tensor_tensor(
                    st8[:, t * n_cold * P : (t + 1) * n_cold * P].rearrange(
                        "p (j m) -> p j m", j=n_cold
                    ),
                    id_c, 1.0, v_c, mb.AluOpType.mult, mb.AluOpType.mult,
                )
                hbs = [
                    ppool.tile([P, bw], f32, tag="ph", name=f"ph_{t}_{b}")
                    for b in range(nb)
                ]
                cbs = [
                    ppool.tile([P, bw], f32, tag="pc", name=f"pc_{t}_{b}")
                    for b in range(nb)
                ]
                # loads in consumption order: hot ranks 0..1, then cold 0..5
                hts, cts = [], []
                for j in range(N_HOT):
                    r = (t * N_HOT + j) * P
                    xt = hpool.tile([P, b_shard], f16, tag="xh", name=f"xh_{t}_{j}")
                    nc.sync.dma_start(out=xt[:], in_=xh_d[r : r + P, :])
                    hts.append(xt)
                for j in range(n_cold):
                    r = (t * n_cold + j) * P
                    xt = cxpool.tile([P, b_shard], f8, tag="xc", name=f"xc_{t}_{j}")
                    nc.sync.dma_start(out=xt[:], in_=xc_d[r : r + P, :])
                    cts.append(xt)
                # hot accumulation (fp16 x fp16), 2 matmuls per bank
                for j in range(N_HOT):
                    hc = t * N_HOT + j
                    for b in range(nb):
                        mm = nc.tensor.matmul(
                            hbs[b][:],
                            lhsT=st16[:, hc * P : (hc + 1) * P],
                            rhs=hts[j][:, b * bw : (b + 1) * bw],
                            start=(j == 0),
                            stop=(j == N_HOT - 1),
                        )
                        if b > 0:
                            # stationary already in the PE array from b=0:
                            # skip the redundant 128-cycle weight reload
                            mm.ins.ldweights = False
                # evacuate hot banks early to fp32 staging (frees them for
                # the next group while cold matmuls still run)
                ysb = spool.tile([P, b_shard], f32, tag="yh", name=f"yh_{t}")
                for b in range(nb):
                    nc.scalar.copy(out=ysb[:, b * bw : (b + 1) * bw], in_=hbs[b][:])
                # cold accumulation (fp8 x fp8), 6 matmuls per bank
                for j in range(n_cold):
                    cc = t * n_cold + j
                    for b in range(nb):
                        mm = nc.tensor.matmul(
                            cbs[b][:],
                            lhsT=st8[:, cc * P : (cc + 1) * P],
                            rhs=cts[j][:, b * bw : (b + 1) * bw],
                            start=(j == 0),
                            stop=(j == n_cold - 1),
                        )
                        if b > 0:
                            mm.ins.ldweights = False
                yt = ypool.tile([P, b_shard], f16, tag="y", name=f"y_{t}")
                if last:
                    # tail: fuse straight from PSUM per bank (no staging
                    # copy), store each half as soon as its banks fuse
                    for b in range(nb):
                        nc.vector.scalar_tensor_tensor(
                            yt[:, b * bw : (b + 1) * bw],
                            cbs[b][:],
                            1.0,
                            ysb[:, b * bw : (b + 1) * bw],
                            mb.AluOpType.mult,
                            mb.AluOpType.add,
                        )
                        if b == 1:
                            nc.scalar.dma_start(
                                out=y_d[t * P : (t + 1) * P, 0 : 2 * bw],
                                in_=yt[:, 0 : 2 * bw],
                            )
                    nc.scalar.dma_start(
                        out=y_d[t * P : (t + 1) * P, 2 * bw :], in_=yt[:, 2 * bw :]
                    )
                else:
                    # free cold banks fast via ACT copies to fp32 staging;
                    # fuse cold + hot -> fp16 on DVE off the PSUM critical path
                    csb = spool.tile([P, b_shard], f32, tag="yc", name=f"yc_{t}")
                    for b in range(nb):
                        nc.scalar.copy(out=csb[:, b * bw : (b + 1) * bw], in_=cbs[b][:])
                    for b in range(nb):
                        nc.vector.scalar_tensor_tensor(
                            yt[:, b * bw : (b + 1) * bw],
                            csb[:, b * bw : (b + 1) * bw],
                            1.0,
                            ysb[:, b * bw : (b + 1) * bw],
                            mb.AluOpType.mult,
                            mb.AluOpType.add,
                        )
                    nc.scalar.dma_start(out=y_d[t * P : (t + 1) * P, :], in_=yt[:])
    nc.compile()
    return nc


def _prep_inputs(x, log_weight, b_shard):
    """Host-side: per-output weight sort, permutation gather, mixed casts."""
    from concourse import mybir

    f8np = mybir.dt.np(mybir.dt.float8e4)
    w = np.exp(np.asarray(log_weight, np.float64)).astype(np.float32)  # [1024, 8]
    ordk = np.argsort(-w, axis=1)  # [1024, 8] feature ranks per output
    o_all = np.arange(OUT_F)
    wsort = w[o_all[:, None], ordk]  # [1024, 8]
    # v[p, 8t+j] = wsort[128t+p, j]
    v = np.ascontiguousarray(
        wsort.reshape(8, P, 8).transpose(1, 0, 2).reshape(P, 64), dtype=np.float32
    )
    feat = (8 * o_all[:, None] + ordk).reshape(8, P, 8)  # [t, p, j]
    hot_idx = feat[:, :, :N_HOT].transpose(0, 2, 1).reshape(-1)  # [(t*2+j)*128+p]
    cold_idx = feat[:, :, N_HOT:].transpose(0, 2, 1).reshape(-1)
    ident = np.eye(P, dtype=np.float16)

    x16 = np.asarray(x, np.float32).astype(np.float16)
    in_maps = []
    for i in range(N_CORES):
        xt = np.ascontiguousarray(x16[i * b_shard : (i + 1) * b_shard].T)
        xh = np.ascontiguousarray(xt[hot_idx])
        xc = np.ascontiguousarray(xt[cold_idx]).astype(f8np)
        in_maps.append({"xh": xh, "xc": xc, "v": v, "idm": ident})
    return in_maps


def kernel(x, log_weight, _trace_dir=None):
    from concourse import bass_utils

    b_shard = B // N_CORES
    if "nc" not in _CACHE:
        _CACHE["nc"] = _build(b_shard, IN_F, OUT_F, N_CORES)
    nc = _CACHE["nc"]

    in_maps = _prep_inputs(x, log_weight, b_shard)
    kwargs = {}
    if _trace_dir is not None:
        kwargs = {"trace": True, "tmpdir": _trace_dir}
    res = bass_utils.run_bass_kernel_spmd(
        nc, in_maps, core_ids=list(range(N_CORES)), **kwargs
    )
    _CACHE["last_res"] = res
    y = np.empty((B, OUT_F), np.float32)
    for i in range(N_CORES):
        y[i * b_shard : (i + 1) * b_shard] = res.results[i]["y"].T.astype(np.float32)
    return y
